# revision 9
# baseline (speedup 1.0000x reference)
"""Trainium2 Bass kernel for XCA-style cross-covariance attention (v3).

Gram-matrix reformulation (single pass over tokens + tiny mid phase +
store-bound output pass):
    S11 = x1^T x1, S21 = x2^T x1, S22 = x2^T x2             # Gram matrices
    nq2[c] = sum_m Aq[m,c] * (S11 Aq)[m,c]   (Aq = Wq.T)
    nk2[c] likewise from S22, Ak
    T2 = S21^T @ Ak ; T2 *= (1/nk)[cols]
    G_h = Aq[:,hb]^T T2[:,hb] ; logits = G_h * (temp/nq)[rows]
    attn_h = softmax(logits);  M[hb,:] = attn_h^T @ Wo.T[hb,:]
    W_eff = Wv.T @ M ;  out = x2 @ W_eff + bo

Schedule:
  - phase 1 is load-bound: x1 and x2 fully SBUF-resident, interleaved
    4-chunk load batches at the modeled DMA roofline; the PE does the 6
    Gram matmuls per chunk plus an f32r transpose pair for every OTHER
    chunk (phase-2 lhsT, stored bf16).  Weight prep is injected at chunk
    16.  A 2-batch backlog delay gives the PE a single p-state ramp.
  - mid phase: engine-aware short chains. GPSIMD cannot touch PSUM, so
    all PSUM reads are on DVE/Act; Act-written matmul operands are f32
    tiles bitcast to f32r at the use site. Norms use the one-op
    Abs_reciprocal_sqrt activation; its table is pre-warmed at t~0 and
    the single reload to the exp table is triggered by a dummy Exp so it
    hides behind the T2-scale/G chain. Explicit Act-queue chaining stops
    the tile scheduler from interleaving sqrt/exp (double reloads).
  - phase 2 is store-bound (4-chunk stores, 4-deep staging): per chunk
    two bf16 matmuls; even chunks: DVE fused add (psum+bias -> staging);
    odd chunks: Act copy psum->staging then Pool in-place bias add
    (SBUF only); deferred transposes for odd chunks flow through a
    6-slot bf16 ring (PE transpose + DVE/Act copy).

Sharding: data-parallel over batch B=8 -> 8 NeuronCores, one batch each.
"""

import os
import sys

import numpy as np

_B, _N, _C, _H = 8, 8192, 256, 4
_P = 128  # SBUF partitions


def _ensure_paths():
    for p in ("/root/.axon_site/_ro/trn_rl_repo", "/opt/trn_rl_repo",
              "/root/.axon_site", "/root/.axon_site/_ro/pypackages"):
        if os.path.isdir(p) and p not in sys.path:
            sys.path.append(p)


def build_nc(n_tokens=_N):
    """Build the single-core Bass program (same program SPMD on 8 cores)."""
    _ensure_paths()
    import concourse.bass as bass
    import concourse.mybir as mybir
    import concourse.tile as tile
    from concourse import bacc
    from concourse.masks import make_identity
    from concourse.tile_rust import add_dep_helper

    f32 = mybir.dt.float32
    f32r = mybir.dt.float32r
    bf16 = mybir.dt.bfloat16
    Exp = mybir.ActivationFunctionType.Exp
    AbsRsqrt = mybir.ActivationFunctionType.Abs_reciprocal_sqrt

    N, C, H = n_tokens, _C, _H
    P = _P
    NCH = N // P          # token chunks of 128
    CT = C // P           # channel tiles (2)
    GB = 4                # chunks per load-DMA batch
    NB = NCH // GB        # load batches per input
    OB = 4                # chunks per store quartet
    RING = 6              # deferred-transpose ring depth

    nc = bacc.Bacc("TRN2", target_bir_lowering=False, debug=False)

    x1_d = nc.dram_tensor("x1", [N, C], f32, kind="ExternalInput").ap()
    x2_d = nc.dram_tensor("x2", [N, C], f32, kind="ExternalInput").ap()
    wq_d = nc.dram_tensor("Wq", [C, C], f32, kind="ExternalInput").ap()
    wk_d = nc.dram_tensor("Wk", [C, C], f32, kind="ExternalInput").ap()
    wv_d = nc.dram_tensor("Wv", [C, C], f32, kind="ExternalInput").ap()
    wo_d = nc.dram_tensor("Wo", [C, C], f32, kind="ExternalInput").ap()
    bo_d = nc.dram_tensor("bo", [C], f32, kind="ExternalInput").ap()
    tp_d = nc.dram_tensor("temperature", [H, 1, 1], f32, kind="ExternalInput").ap()
    out_d = nc.dram_tensor("out", [N, C], f32, kind="ExternalOutput").ap()

    def r(ap):
        return ap.bitcast(f32r)

    with tile.TileContext(nc) as tc:
        with tc.tile_pool(name="consts", bufs=1) as consts, \
             tc.tile_pool(name="work", bufs=1, space="PSUM") as work:
            opsum_cm = tc.tile_pool(name="opsum", bufs=3, space="PSUM")
            opsum = opsum_cm.__enter__()

            ident = consts.tile([P, P], f32, name="ident", tag="ident")
            make_identity(nc, ident)
            ident_r = consts.tile([P, P], f32r, name="ident_r", tag="ident_r")
            nc.vector.tensor_copy(ident_r, ident)
            ident_b = consts.tile([P, P], bf16, name="ident_b", tag="ident_b")
            nc.vector.tensor_copy(ident_b, ident)
            ones_f = consts.tile([P, P + 1], f32, name="ones_f", tag="ones_f")
            nc.vector.memset(ones_f, 1.0)
            ones_red = consts.tile([P, 1], f32r, name="ones_red", tag="ones_red")
            nc.vector.tensor_copy(ones_red, ones_f[:, 0:1])
            ones_bf = consts.tile([1, P], bf16, name="ones_bf", tag="ones_bf")
            nc.vector.tensor_copy(ones_bf, ones_f[0:1, 0:P])
            # pre-warm abs_reciprocal_sqrt_and_small (has rsqrt + copy)
            scrap = consts.tile([1, 4], f32, name="scrap", tag="scrap")
            nc.scalar.activation(scrap[0:1, 1:2], ones_f[0:1, 0:1], AbsRsqrt)

            # ---- big input staging (both inputs fully resident) ----
            x1s = consts.tile([P, NCH, C], f32r, name="x1s", tag="x1s")
            x2s = consts.tile([P, NCH, C], f32r, name="x2s", tag="x2s")
            # transposed x2 (bf16): every other chunk persistent, rest ring
            x2te = consts.tile([P, CT, (NCH // 2) * P], bf16, name="x2te",
                               tag="x2te")
            x2tr = consts.tile([P, CT, RING, P], bf16, name="x2tr", tag="x2tr")
            xbf = consts.tile([P, 8, C], bf16, name="xbf", tag="xbf")
            deferred = [i for i in range(NCH) if i % 2 == 1]
            didx = {i: k for k, i in enumerate(deferred)}

            # ---- natural-layout weights ----
            wq_n = consts.tile([P, CT, C], f32, name="wq_n", tag="wq_n")
            wk_n = consts.tile([P, CT, C], f32, name="wk_n", tag="wk_n")
            wv_n = consts.tile([P, CT, C], f32, name="wv_n", tag="wv_n")
            wo_n = consts.tile([P, CT, C], f32, name="wo_n", tag="wo_n")
            bo_f = consts.tile([1, C], f32, name="bo_f", tag="bo_f")
            tempsb = consts.tile([1, H], f32, name="tempsb", tag="tempsb")

            # ---- load stream (SP queue) ----
            def load_batch(dram, dst, g):
                srcp = bass.AP(tensor=dram.tensor,
                               offset=dram.offset + g * GB * P * C,
                               ap=[[C, P], [P * C, GB], [1, C]]).bitcast(f32r)
                return nc.sync.dma_start(dst[:, g * GB:(g + 1) * GB, :], srcp)

            load_batch(x2_d, x2s, 0)
            load_batch(x1_d, x1s, 0)
            load_batch(x2_d, x2s, 1)
            load_batch(x1_d, x1s, 1)
            load_batch(x2_d, x2s, 2)
            x1_backlog = load_batch(x1_d, x1s, 2)
            for (wd, wn) in ((wq_d, wq_n), (wk_d, wk_n), (wv_d, wv_n),
                             (wo_d, wo_n)):
                srcp = bass.AP(tensor=wd.tensor, offset=wd.offset,
                               ap=[[C, P], [P * C, CT], [1, C]])
                nc.sync.dma_start(wn, srcp)
            for g in range(3, NB):
                load_batch(x2_d, x2s, g)
                load_batch(x1_d, x1s, g)
            # tiny mid-phase-only loads at the tail of the stream
            nc.sync.dma_start(bo_f, bo_d.partition_broadcast(1))
            nc.sync.dma_start(tempsb, bass.AP(
                tensor=tp_d.tensor, offset=tp_d.offset, ap=[[0, 1], [1, H]]))

            # transposed weights Aq=Wq.T, Ak=Wk.T, Ao=Wo.T (f32 tiles,
            # bitcast to f32r at the matmul operand)
            aq = consts.tile([P, CT, C], f32r, name="aq", tag="aq")
            ak = consts.tile([P, CT, C], f32r, name="ak", tag="ak")
            ao = consts.tile([P, CT, C], f32r, name="ao", tag="ao")
            wv_r = consts.tile([P, CT, C], f32r, name="wv_r", tag="wv_r")
            ao_bf = consts.tile([P, CT, C], bf16, name="ao_bf", tag="ao_bf")
            bob2 = consts.tile([P, 2, C], f32, name="bob2", tag="bob2")
            tempflat = consts.tile([1, C], f32, name="tempflat", tag="tempflat")
            tempcol = [consts.tile([P, 1], f32, name=f"tc{t}", tag=f"tc{t}")
                       for t in range(CT)]

            def weight_prep():
                # PE transposes; PSUM->SBUF copies on Act (idle in phase 1)
                for (nat, tr) in ((wq_n, aq), (wk_n, ak), (wo_n, ao)):
                    for ti in range(CT):
                        tpw = work.tile([P, C], f32, name="tp", tag="tp",
                                        bufs=2)
                        for tj in range(CT):
                            nc.tensor.transpose(
                                tpw[:, tj * P:(tj + 1) * P],
                                nat[:, tj, ti * P:(ti + 1) * P], ident)
                        nc.vector.tensor_copy(tr[:, ti, :], tpw)
                        if nat is wo_n:
                            nc.scalar.copy(ao_bf[:, ti, :], tpw)
                nc.vector.tensor_copy(wv_r, wv_n)

            # ---- phase 1: Gram accumulation ----
            gpsum_cm = tc.tile_pool(name="gpsum", bufs=1, space="PSUM")
            gpsum = gpsum_cm.__enter__()
            s11p = gpsum.tile([P, 2 * C], f32, name="s11", tag="s11")
            s21p = gpsum.tile([P, 2 * C], f32, name="s21", tag="s21")
            s22p = gpsum.tile([P, 2 * C], f32, name="s22", tag="s22")

            def transpose_pair(i, conv_eng, pool=None, bufs=2, tag="tp"):
                """Convert x2 chunk i to bf16, PE-transpose into PSUM."""
                slot = (i // 2) % 8 if i % 2 == 0 else didx[i] % 8
                conv_eng(xbf[:, slot, :], x2s[:, i, :].bitcast(f32))
                tp2 = (pool or work).tile([P, C], bf16, name=tag, tag=tag,
                                          bufs=bufs)
                for t in range(CT):
                    nc.tensor.transpose(
                        tp2[:, t * P:(t + 1) * P],
                        xbf[:, slot, t * P:(t + 1) * P], ident_b)
                return tp2

            def ring_copy(i, tp2, eng, chain_fn=None):
                inst = eng(
                    x2tr[:, :, didx[i] % RING, :],
                    tp2.rearrange("p (t q) -> p t q", t=CT))
                if chain_fn is not None:
                    chain_fn(inst)
                return inst

            for i in range(NCH):
                x1c = x1s[:, i, :]
                x2c = x2s[:, i, :]
                sp = (i == NCH - 1)
                for t in range(CT):
                    st = (i == 0) and (t == 0)
                    mm = nc.tensor.matmul(
                        s22p[:, t * C:(t + 1) * C], x2c[:, t * P:(t + 1) * P],
                        x2c, start=st, stop=sp, skip_group_check=True)
                    if i == 0 and t == 0:
                        # hold PE until a 2-batch backlog is banked so the
                        # Gram stream runs gap-free (single p-state ramp)
                        add_dep_helper(mm.ins, x1_backlog.ins, True,
                                       "PE backlog delay")
                for t in range(CT):
                    st = (i == 0) and (t == 0)
                    nc.tensor.matmul(
                        s11p[:, t * C:(t + 1) * C], x1c[:, t * P:(t + 1) * P],
                        x1c, start=st, stop=sp, skip_group_check=True)
                for t in range(CT):
                    st = (i == 0) and (t == 0)
                    nc.tensor.matmul(
                        s21p[:, t * C:(t + 1) * C], x2c[:, t * P:(t + 1) * P],
                        x1c, start=st, stop=sp, skip_group_check=True)
                if i % 2 == 0:
                    j = i // 2
                    tp2 = transpose_pair(i, nc.vector.tensor_copy)
                    # keep the DVE free near the phase boundary: the last
                    # few transposed-chunk copies ride on Act instead
                    ceng = (nc.scalar.copy if i >= NCH - 8
                            else nc.vector.tensor_copy)
                    ceng(x2te[:, :, j * P:(j + 1) * P],
                         tp2.rearrange("p (t q) -> p t q", t=CT))
                if i == 16:
                    weight_prep()

            # ---- mid phase ----
            s11 = consts.tile([P, CT, C], f32r, name="s11s", tag="s11s")
            s21 = consts.tile([P, CT, C], f32r, name="s21s", tag="s21s")
            s22 = consts.tile([P, CT, C], f32r, name="s22s", tag="s22s")
            act_chain = []

            def chain(inst):
                if act_chain:
                    add_dep_helper(inst.ins, act_chain[-1].ins, True,
                                   "act order")
                act_chain.append(inst)
                return inst

            for t in range(CT):
                nc.vector.tensor_copy(s22[:, t, :], s22p[:, t * C:(t + 1) * C])
            for t in range(CT):
                nc.vector.tensor_copy(s21[:, t, :], s21p[:, t * C:(t + 1) * C])
            for t in range(CT):
                nc.vector.tensor_copy(s11[:, t, :], s11p[:, t * C:(t + 1) * C])
            gpsum_cm.__exit__(None, None, None)
            # recycle the freed gram banks: ring-prefill transposes + the
            # fixed "small" bank for norm/bias broadcasts
            prefill_cm = tc.tile_pool(name="prefill", bufs=1, space="PSUM")
            prefill = prefill_cm.__enter__()

            # temperature -> flat per-channel row, then per-tile columns
            # (tempsb arrives at the tail of the load stream)
            for h in range(H):
                nc.vector.tensor_scalar_mul(
                    tempflat[0:1, h * (C // H):(h + 1) * (C // H)],
                    ones_f[0:1, 0:C // H], tempsb[0:1, h:h + 1])
            for t in range(CT):
                nc.scalar.dma_start(tempcol[t],
                                    tempflat[0:1, t * P:(t + 1) * P])

            # bias broadcast early (first opsum slot, consumer is cheap)
            bobp = opsum.tile([P, C], f32, name="m", tag="o")
            nc.tensor.matmul(bobp, ones_f[0:1, 0:P], bo_f,
                             start=True, stop=True, skip_group_check=True)
            nc.vector.tensor_copy(bob2[:, 0, :], bobp)
            nc.vector.tensor_copy(bob2[:, 1, :], bobp)

            # k-norm chain: u22 = S22 Ak ; vvk = Ak .* u22 ; nk2 = ones^T vvk
            vvk = consts.tile([P, CT, C], f32r, name="vvk", tag="vvk")
            vvq = consts.tile([P, CT, C], f32, name="vvq", tag="vvq")
            u22t = []
            for t in range(CT):
                u = opsum.tile([P, C], f32, name="m", tag="o")
                for uu in range(CT):
                    nc.tensor.matmul(
                        u, s22[:, uu, t * P:(t + 1) * P], ak[:, uu, :],
                        start=(uu == 0), stop=(uu == CT - 1),
                        skip_group_check=True)
                u22t.append(u)
            # T2 = S12 @ Ak (unscaled; k-norm applied to T2 columns later)
            t2p = []
            for t in range(CT):
                tp_ = opsum.tile([P, C], f32, name="m", tag="o")
                for uu in range(CT):
                    nc.tensor.matmul(
                        tp_, s21[:, uu, t * P:(t + 1) * P], ak[:, uu, :],
                        start=(uu == 0), stop=(uu == CT - 1),
                        skip_group_check=True)
                t2p.append(tp_)
            nc.vector.tensor_mul(vvk[:, 0, :], ak[:, 0, :], u22t[0])
            nc.vector.tensor_mul(vvk[:, 1, :], ak[:, 1, :], u22t[1])

            # nk2 flat row [1, C] -- emitted before the q-side so the PE
            # reaches it as soon as vvk lands (it gates the whole k chain)
            nfk = opsum.tile([1, C], f32, name="m", tag="o")
            for t in range(CT):
                nc.tensor.matmul(nfk, ones_red, vvk[:, t, :],
                                 start=(t == 0), stop=(t == CT - 1),
                                 skip_group_check=True)

            # q-side: uq = S11 Aq ; vvq = Aq .* uq
            uqt = []
            for t in range(CT):
                u = opsum.tile([P, C], f32, name="m", tag="o")
                for uu in range(CT):
                    nc.tensor.matmul(
                        u, s11[:, uu, t * P:(t + 1) * P], aq[:, uu, :],
                        start=(uu == 0), stop=(uu == CT - 1),
                        skip_group_check=True)
                uqt.append(u)
            nc.vector.tensor_mul(vvq[:, 0, :], aq[:, 0, :].bitcast(f32), uqt[0])
            nc.vector.tensor_mul(vvq[:, 1, :], aq[:, 1, :].bitcast(f32), uqt[1])
            # nq2 as per-partition columns in one FIXED psum bank together
            # with bnk (avoids opsum-rotation cross deps). First writer
            # zeroes the whole bank.
            small = prefill.tile([P, 512], f32, name="small", tag="small")
            nqp = []
            nqp_first = None
            for t2 in range(CT):
                u = small[:, 256 + t2:257 + t2]
                for t in range(CT):
                    mm = nc.tensor.matmul(
                        u, vvq[:, t, t2 * P:(t2 + 1) * P], ones_f[:, 0:1],
                        start=(t2 == 0 and t == 0), stop=(t == CT - 1),
                        skip_group_check=True)
                    if nqp_first is None:
                        nqp_first = mm
                nqp.append(u)

            nk_inv = consts.tile([1, C], bf16, name="nk_inv", tag="nk_inv")
            chain(nc.scalar.activation(nk_inv, nfk, AbsRsqrt))

            # PE-gap work: first ring prefills
            pref_tiles = {}
            for i in deferred[:2]:
                pref_tiles[i] = transpose_pair(i, nc.gpsimd.tensor_copy,
                                               pool=prefill, bufs=2, tag="pf")

            # bnk = broadcast of nk_inv over partitions into "small"
            bnkp = small[:, 0:256]
            bnk_mm = nc.tensor.matmul(bnkp, ones_bf, nk_inv,
                                      start=False, stop=True,
                                      skip_group_check=True)
            add_dep_helper(bnk_mm.ins, nqp_first.ins, True, "small bank zero")
            bnk_sb = consts.tile([P, C], f32, name="bnk_sb", tag="bnk_sb")
            chain(nc.scalar.copy(bnk_sb, bnkp))

            # rowscale[t2] = temp / nq  as [P, 1] columns
            rowscale = []
            for t2 in range(CT):
                iv = consts.tile([P, 1], f32, name=f"iv{t2}", tag=f"iv{t2}")
                chain(nc.scalar.activation(iv, nqp[t2], AbsRsqrt))
                rs = consts.tile([P, 1], f32, name=f"rs{t2}", tag=f"rs{t2}")
                nc.vector.tensor_mul(rs, iv, tempcol[t2])
                rowscale.append(rs)

            # dummy Exp: trigger the rsqrt->exp table reload NOW so it hides
            # behind the t2s/G chain instead of stalling the real Exp
            chain(nc.scalar.activation(scrap[0:1, 2:3], ones_f[0:1, 0:1], Exp))

            # t2s = T2 .* (1/nk)[cols]
            t2s = consts.tile([P, CT, C], f32r, name="t2s", tag="t2s")
            nc.vector.tensor_mul(t2s[:, 0, :], t2p[0], bnk_sb)
            nc.vector.tensor_mul(t2s[:, 1, :], t2p[1], bnk_sb)

            # G pairs + softmax + M + W_eff
            mm_sb = consts.tile([P, CT, C], f32r, name="mm_sb", tag="mm_sb")
            weff = consts.tile([P, CT, C], bf16, name="weff", tag="weff")
            for t in range(2):  # head pair (2t, 2t+1)
                g2 = opsum.tile([P, 64], f32, name="m", tag="o")
                for par in range(2):
                    h = 2 * t + par
                    hb = slice(h * 64, (h + 1) * 64)
                    for uu in range(CT):
                        nc.tensor.matmul(
                            g2[par * 64:(par + 1) * 64, :],
                            aq[:, uu, hb].bitcast(f32), t2s[:, uu, hb].bitcast(f32),
                            start=(uu == 0), stop=(uu == CT - 1),
                            skip_group_check=True)
                ex = consts.tile([P, 64], f32, name=f"ex{t}", tag=f"ex{t}")
                sume = consts.tile([P, 1], f32, name=f"se{t}", tag=f"se{t}")
                chain(nc.scalar.activation(ex, g2, Exp, scale=rowscale[t],
                                           accum_out=sume))
                sinv = consts.tile([P, 1], f32, name=f"si{t}", tag=f"si{t}")
                nc.vector.reciprocal(sinv, sume)
                at2 = consts.tile([P, 64], bf16, name=f"at{t}", tag=f"at{t}")
                nc.vector.tensor_scalar_mul(at2, ex, sinv)

                mmp = opsum.tile([P, C], f32, name="m", tag="o")
                for par in range(2):
                    sl = slice(par * 64, (par + 1) * 64)
                    nc.tensor.matmul(
                        mmp[sl, :], at2[sl, :], ao_bf[sl, t, :],
                        start=True, stop=True, skip_group_check=True)
                nc.vector.tensor_copy(mm_sb[:, t, :], mmp)

            for t in range(CT):
                wp = opsum.tile([P, C], f32, name="m", tag="o")
                for uu in range(CT):
                    nc.tensor.matmul(
                        wp, wv_r[:, uu, t * P:(t + 1) * P], mm_sb[:, uu, :],
                        start=(uu == 0), stop=(uu == CT - 1),
                        skip_group_check=True)
                nc.vector.tensor_copy(weff[:, t, :], wp)

            # second half of the ring prefill; copies split DVE/Act (the
            # Act ones chained after the Exps)
            for i in deferred[2:RING]:
                pref_tiles[i] = transpose_pair(i, nc.gpsimd.tensor_copy,
                                               pool=prefill, bufs=2, tag="pf")
            for k, i in enumerate(deferred[:RING]):
                if k % 2 == 0:
                    ring_copy(i, pref_tiles[i], nc.vector.tensor_copy)
                else:
                    ring_copy(i, pref_tiles[i], nc.scalar.copy,
                              chain_fn=chain)

            # ---- phase 2: out = x2 @ W_eff + bo ----
            # 2 chunks share one PSUM bank (only the very first matmul of a
            # pair starts; bank-wide pending-zero covers the second chunk),
            # so ONE DVE add handles a whole pair: 329 ns/chunk < the
            # 364 ns/chunk store pace. Ring copies ride on Act.
            prefill_cm.__exit__(None, None, None)
            opsum_cm.__exit__(None, None, None)
            p2sum_cm = tc.tile_pool(name="p2sum", bufs=6, space="PSUM")
            p2sum = p2sum_cm.__enter__()
            ostr = consts.tile([P, 4, OB, C], f32, name="ostr", tag="ostr")
            ops2 = None
            for i in range(NCH):
                q = (i // OB) % 4
                if i % 2 == 0:
                    ops2 = p2sum.tile([P, 2, C], f32, name="o2", tag="o2")
                ops = ops2[:, i % 2, :]
                for t in range(CT):
                    if i % 2 == 0:
                        lhs = x2te[:, t, (i // 2) * P:(i // 2 + 1) * P]
                    else:
                        lhs = x2tr[:, t, didx[i] % RING, :]
                    nc.tensor.matmul(ops, lhs, weff[:, t, :],
                                     start=(i % 2 == 0 and t == 0),
                                     stop=(i % 2 == 1 and t == CT - 1),
                                     skip_group_check=True)
                if i % 2 == 1:
                    # one fused psum+bias add for the whole pair
                    nc.vector.tensor_add(
                        ostr[:, q, i % OB - 1:i % OB + 1, :], ops2, bob2)
                    k = didx[i]
                    if k + RING < len(deferred):
                        nxt = deferred[k + RING]
                        ring_copy(nxt,
                                  transpose_pair(nxt, nc.gpsimd.tensor_copy),
                                  nc.scalar.copy)
                if i < OB and i % 2 == 1:
                    # first quartet ships as two half stores so the store
                    # train starts one pair-add earlier
                    dst = bass.AP(
                        tensor=out_d.tensor,
                        offset=out_d.offset + (i - 1) * P * C,
                        ap=[[C, P], [P * C, 2], [1, C]])
                    nc.sync.dma_start(dst, ostr[:, q, i - 1:i + 1, :])
                elif i >= OB and i % OB == OB - 1:
                    b0 = i - OB + 1
                    dst = bass.AP(
                        tensor=out_d.tensor,
                        offset=out_d.offset + b0 * P * C,
                        ap=[[C, P], [P * C, OB], [1, C]])
                    nc.sync.dma_start(dst, ostr[:, q, :, :])
            p2sum_cm.__exit__(None, None, None)

    nc.compile()
    return nc


_NC_CACHE = {}


def _get_nc(n_tokens=_N):
    if n_tokens not in _NC_CACHE:
        _NC_CACHE[n_tokens] = build_nc(n_tokens)
    return _NC_CACHE[n_tokens]


def kernel(x1, x2, Wq, Wk, Wv, Wo, bo, temperature):
    _ensure_paths()
    from concourse.bass_utils import run_bass_kernel_spmd

    B = x1.shape[0]
    nc = _get_nc(x1.shape[1])
    in_maps = []
    for b in range(B):
        in_maps.append({
            "x1": np.ascontiguousarray(x1[b], dtype=np.float32),
            "x2": np.ascontiguousarray(x2[b], dtype=np.float32),
            "Wq": np.asarray(Wq, dtype=np.float32),
            "Wk": np.asarray(Wk, dtype=np.float32),
            "Wv": np.asarray(Wv, dtype=np.float32),
            "Wo": np.asarray(Wo, dtype=np.float32),
            "bo": np.asarray(bo, dtype=np.float32),
            "temperature": np.asarray(temperature, dtype=np.float32),
        })
    res = run_bass_kernel_spmd(nc, in_maps, core_ids=list(range(B)))
    return np.stack([res.results[b]["out"] for b in range(B)]).astype(np.float32)
